# revision 6
# baseline (speedup 1.0000x reference)
"""LocalInfoNCE loss on 8 trn2 cores.

Strategy (data-parallel over batch, per sharding hint):
  - Each core owns BS/8 = 2 output batch elements.
  - Host regroups the (region-major) gather indices per core into flat row
    offsets, and ships each core the f1/f2 batches its offsets reference
    (with the real index structure that is exactly its own 2 batches).
  - Device kernel: indirect-DMA gather of 468 rows x 64ch, PE transpose to
    channel-on-partition layout, per-batch gram matrix S = p @ p.T via 9
    accumulating matmuls (K=64 per pixel), then the InfoNCE epilogue on
    26x26 tiles:  loss_i = log(sum_{j!=i} exp(sim_ij)) - sim_{i,pos(i)}
    where sim = S * rs_i * rs_j / tau and rs_i = 1/max(sqrt(S_ii), eps).
  - Host averages the 8x52 per-row losses (the only cross-core reduction).
"""

import numpy as np

BS, H, W, C = 16, 192, 192, 64
R = 13
KK = 9
TWO_R = 2 * R
TAU = 0.5
EPS = 1e-8
NCORES = 8
BPC = BS // NCORES            # batches per core = 2
ROWS_PC = BPC * TWO_R * KK    # 468 gather rows per core
GCH = (ROWS_PC + 127) // 128  # gather chunks of 128 rows = 4

_prog_cache = {}
LAST_RESULT = None


def _build(nb):
    """Build the SPMD bass program for `nb` shipped batches per feature."""
    from concourse import bass, bacc, mybir
    from concourse.tile import TileContext
    from concourse.masks import make_identity

    f32 = mybir.dt.float32
    i32 = mybir.dt.int32
    Alu = mybir.AluOpType
    Act = mybir.ActivationFunctionType

    nc = bacc.Bacc(None, target_bir_lowering=False, debug=False)
    nrows = 2 * nb * H * W
    fsh = nc.dram_tensor("fsh", [nrows, C], f32, kind="ExternalInput")
    offs = nc.dram_tensor("offs", [GCH, 128], i32, kind="ExternalInput")
    lout = nc.dram_tensor("lout", [BPC, TWO_R], f32, kind="ExternalOutput")

    with TileContext(nc) as tc:
        with (
            tc.tile_pool(name="cpool", bufs=1) as cpool,
            tc.tile_pool(name="pool", bufs=2) as pool,
            tc.tile_pool(name="ppool", bufs=2, space="PSUM") as ppool,
        ):
            ident = cpool.tile([128, 128], f32)
            make_identity(nc, ident)
            i26 = ident[0:TWO_R, 0:TWO_R]
            # pos-pair permutation mask: ppos[i, (i+R) % 2R] = 1
            ppos = cpool.tile([TWO_R, TWO_R], f32)
            nc.vector.tensor_copy(ppos[:, 0:R], ident[0:TWO_R, R:TWO_R])
            nc.vector.tensor_copy(ppos[:, R:TWO_R], ident[0:TWO_R, 0:R])

            offs_t = cpool.tile([128, GCH], i32)
            nc.sync.dma_start(out=offs_t[:, :], in_=offs[:, :].rearrange("c p -> p c"))

            # gather: 128 rows of 64 contiguous floats per chunk
            rows = pool.tile([128, GCH * C], f32)
            for ch in range(GCH):
                nc.gpsimd.indirect_dma_start(
                    out=rows[:, ch * C:(ch + 1) * C],
                    out_offset=None,
                    in_=fsh[:, :],
                    in_offset=bass.IndirectOffsetOnAxis(
                        ap=offs_t[:, ch:ch + 1], axis=0
                    ),
                )

            # transpose to channel-on-partition: G[64, g] = rows[g, ch]
            # single PSUM tile (one bank) so G has a single producer copy
            G = pool.tile([64, GCH * 128], f32)
            tp = ppool.tile([64, GCH * 128], f32, tag="tp")
            for ch in range(GCH):
                nc.tensor.transpose(
                    out=tp[:, ch * 128:(ch + 1) * 128],
                    in_=rows[:, ch * C:(ch + 1) * C],
                    identity=ident,
                )
            nc.vector.tensor_copy(G[:, :], tp[:, :])

            lossv = pool.tile([TWO_R, BPC], f32)
            for bl in range(BPC):
                S = ppool.tile([TWO_R, TWO_R], f32, tag="S")
                for pix in range(KK):
                    a = G[:, (bl * KK + pix) * TWO_R:(bl * KK + pix + 1) * TWO_R]
                    nc.tensor.matmul(
                        out=S[:, :], lhsT=a, rhs=a,
                        start=(pix == 0), stop=(pix == KK - 1),
                    )
                # diag -> row norms
                junk = pool.tile([TWO_R, TWO_R], f32, tag="junk")
                d = pool.tile([TWO_R, 1], f32, tag="d")
                nc.vector.tensor_tensor(out=junk, in0=S[:, :], in1=i26, op=Alu.mult)
                nc.vector.reduce_sum(d[:, :], junk[:, :], axis=mybir.AxisListType.X)
                sn = pool.tile([TWO_R, 1], f32, tag="sn")
                nc.scalar.sqrt(sn, d)
                snc = pool.tile([TWO_R, 1], f32, tag="snc")
                nc.vector.tensor_scalar_max(snc, sn, EPS)
                ri = pool.tile([TWO_R, 1], f32, tag="ri")
                nc.vector.reciprocal(ri, snc)
                rs = pool.tile([TWO_R, 1], f32, tag="rs")
                nc.scalar.mul(rs, ri, float(1.0 / np.sqrt(TAU)))
                # sim = rs_i * rs_j * S  (S symmetric: row-scale, transpose, row-scale)
                T1 = pool.tile([TWO_R, TWO_R], f32, tag="T1")
                nc.vector.tensor_scalar_mul(T1, S[:, :], rs)
                T1t = ppool.tile([TWO_R, TWO_R], f32, tag="T1t")
                nc.tensor.transpose(out=T1t[:, :], in_=T1, identity=i26)
                sim = pool.tile([TWO_R, TWO_R], f32, tag="sim")
                nc.vector.tensor_scalar_mul(sim, T1t[:, :], rs)
                # Z_i = sum_{j != i} exp(sim_ij);  loss_i = log(Z_i) - sim_{i,pos}
                E = pool.tile([TWO_R, TWO_R], f32, tag="E")
                rsum = pool.tile([TWO_R, 1], f32, tag="rsum")
                nc.scalar.activation(E, sim, Act.Exp, accum_out=rsum)
                junk2 = pool.tile([TWO_R, TWO_R], f32, tag="junk2")
                dE = pool.tile([TWO_R, 1], f32, tag="dE")
                nc.vector.tensor_tensor(out=junk2, in0=E, in1=i26, op=Alu.mult)
                nc.vector.reduce_sum(dE[:, :], junk2[:, :], axis=mybir.AxisListType.X)
                Z = pool.tile([TWO_R, 1], f32, tag="Z")
                nc.vector.tensor_tensor(out=Z, in0=rsum, in1=dE, op=Alu.subtract)
                L = pool.tile([TWO_R, 1], f32, tag="L")
                nc.scalar.activation(L, Z, Act.Ln)
                junk3 = pool.tile([TWO_R, TWO_R], f32, tag="junk3")
                pos = pool.tile([TWO_R, 1], f32, tag="pos")
                nc.vector.tensor_tensor(out=junk3, in0=sim, in1=ppos, op=Alu.mult)
                nc.vector.reduce_sum(pos[:, :], junk3[:, :], axis=mybir.AxisListType.X)
                nc.vector.tensor_tensor(
                    out=lossv[:, bl:bl + 1], in0=L, in1=pos, op=Alu.subtract
                )
            nc.sync.dma_start(
                out=lout[:, :].rearrange("b r -> r b"), in_=lossv[:, :]
            )
    nc.finalize()
    return nc


def kernel(f1, f2, b_idx, h_idx, w_idx):
    global LAST_RESULT
    from concourse.bass_utils import run_bass_kernel_spmd

    f1 = np.asarray(f1, dtype=np.float32)
    f2 = np.asarray(f2, dtype=np.float32)
    b_idx = np.asarray(b_idx).astype(np.int64)
    h_idx = np.asarray(h_idx).astype(np.int64)
    w_idx = np.asarray(w_idx).astype(np.int64)

    n = R * BS * KK
    j = np.arange(n)
    reg = j // (BS * KK)          # region of gather row j
    bpos = (j // KK) % BS         # positional output batch of row j
    pix = j % KK                  # pixel within block

    # which input batches does each core's gather touch?
    ship = []
    for c in range(NCORES):
        mask = (bpos // BPC) == c
        ship.append(np.unique(b_idx[mask]))
    nb = max(len(s) for s in ship)

    in_maps = []
    for c in range(NCORES):
        sb = ship[c]
        mask = (bpos // BPC) == c
        # local slot of each referenced input batch
        lslot = np.searchsorted(sb, b_idx[mask])
        bl = bpos[mask] % BPC
        px = pix[mask]
        rg = reg[mask]
        base = (lslot * H + h_idx[mask]) * W + w_idx[mask]  # row within one feature block
        offs = np.zeros(GCH * 128, np.int32)
        for s in range(2):  # feature side: f1 rows i<R, f2 rows i>=R
            g = (bl * KK + px) * TWO_R + s * R + rg
            offs[g] = base + s * nb * H * W
        fsh = np.zeros((2, nb, H * W, C), np.float32)
        fsh[0, : len(sb)] = f1[sb].reshape(len(sb), H * W, C)
        fsh[1, : len(sb)] = f2[sb].reshape(len(sb), H * W, C)
        in_maps.append(
            {"fsh": fsh.reshape(2 * nb * H * W, C), "offs": offs.reshape(GCH, 128)}
        )

    key = nb
    if key not in _prog_cache:
        _prog_cache[key] = _build(nb)
    nc = _prog_cache[key]

    LAST_RESULT = run_bass_kernel_spmd(nc, in_maps, list(range(NCORES)))
    lv = np.concatenate([r["lout"].reshape(-1) for r in LAST_RESULT.results])
    return np.float32(lv.mean())


# revision 15
# speedup vs baseline: 1.0893x; 1.0893x over previous
"""LocalInfoNCE loss on 8 trn2 cores.

Strategy (data-parallel over batch, per sharding hint):
  - Each core owns BS/8 = 2 output batch elements.
  - Host regroups the (region-major) gather indices per core into flat row
    offsets, and ships each core the f1/f2 batches its offsets reference
    (with the real index structure that is exactly its own 2 batches).
  - Device kernel: indirect-DMA gather of 468 rows x 64ch (offsets read
    directly from DRAM), PE transpose to channel-on-partition layout,
    per-batch gram matrix S = p @ p.T via 9 accumulating matmuls (K=64 per
    pixel), then one stacked (52, 26) InfoNCE epilogue for both batches:
      loss_i = log(sum_{j!=i} exp(sim_ij)) - sim_{i,pos(i)}
    with sim = S * rs_i * rs_j / tau, rs_i = 1/max(sqrt(S_ii), eps).
  - Host averages the 8x52 per-row losses (the only cross-core reduction).
"""

import numpy as np

BS, H, W, C = 16, 192, 192, 64
R = 13
KK = 9
TWO_R = 2 * R
TAU = 0.5
EPS = 1e-8
NCORES = 8
BPC = BS // NCORES            # batches per core = 2
PB = 32                       # padded per-batch block (PE quad alignment)
NRP = BPC * PB                # stacked padded rows per core = 64
ROWS_PC = BPC * TWO_R * KK    # 468 gather rows per core
GCH = (ROWS_PC + 127) // 128  # gather chunks of 128 rows = 4

_prog_cache = {}
LAST_RESULT = None


def _build(nb):
    """Build the SPMD bass program for `nb` shipped batches per feature."""
    from concourse import bass, bacc, mybir
    from concourse.tile import TileContext
    from concourse.masks import make_identity

    f32 = mybir.dt.float32
    i32 = mybir.dt.int32
    Alu = mybir.AluOpType
    Act = mybir.ActivationFunctionType

    nc = bacc.Bacc(None, target_bir_lowering=False, debug=False)
    nrows = 2 * nb * H * W
    fsh = nc.dram_tensor("fsh", [nrows, C], f32, kind="ExternalInput")
    offs = nc.dram_tensor("offs", [128, GCH], i32, kind="ExternalInput")
    lout = nc.dram_tensor("lout", [NRP, 1], f32, kind="ExternalOutput")

    with TileContext(nc) as tc:
        with (
            tc.tile_pool(name="cpool", bufs=1) as cpool,
            tc.tile_pool(name="pool", bufs=2) as pool,
            tc.tile_pool(name="ppool", bufs=2, space="PSUM") as ppool,
        ):
            # hoist the Sqrt/Exp/Ln activation-table loads off the critical
            # path: touch each function once right at kernel start
            warm = cpool.tile([1, 1], f32)
            nc.vector.memset(warm, 1.0)
            for fn in (Act.Sqrt, Act.Exp, Act.Ln):
                nc.scalar.activation(warm, warm, fn)

            ident = cpool.tile([128, 128], f32)
            make_identity(nc, ident)
            # stacked masks over both batches' padded 32-row blocks (cols 0:26
            # are real, 26:32 padding):
            #  mI[i, j]    = 1 if j == i%32                (diag selector)
            #  mNotI[i, j] = 1 if j < 26 and j != i%32     (logsumexp mask)
            #  mP[i, j]    = 1 if j == (i%32 + R) % 26     (positive selector)
            mIm = cpool.tile([NRP, PB], f32)
            nc.gpsimd.memset(mIm, 0.0)
            mNotI = cpool.tile([NRP, PB], f32)
            nc.gpsimd.memset(mNotI, 0.0)
            nc.gpsimd.memset(mNotI[:, 0:TWO_R], 1.0)
            for bl in range(BPC):
                blk = slice(bl * PB, (bl + 1) * PB)
                nc.gpsimd.affine_select(
                    out=mIm[blk, :], in_=mIm[blk, :],
                    compare_op=Alu.not_equal, fill=1.0,
                    base=0, pattern=[[-1, PB]], channel_multiplier=1,
                )
                nc.gpsimd.affine_select(
                    out=mNotI[blk, :], in_=mNotI[blk, :],
                    compare_op=Alu.not_equal, fill=0.0,
                    base=0, pattern=[[-1, PB]], channel_multiplier=1,
                )
            mP = cpool.tile([NRP, PB], f32)
            nc.gpsimd.memset(mP, 0.0)
            nc.vector.tensor_copy(mP[:, 0:R], mIm[:, R:TWO_R])
            nc.vector.tensor_copy(mP[:, R:TWO_R], mIm[:, 0:R])

            # gather: 128 rows of 64 contiguous floats per chunk (offset
            # table staged to SBUF first -- HW requires SB-resident offsets)
            offs_t = cpool.tile([128, GCH], i32)
            nc.sync.dma_start(out=offs_t[:, :], in_=offs[:, :])
            rows = pool.tile([128, GCH * C], f32)
            for ch in range(GCH):
                nc.gpsimd.indirect_dma_start(
                    out=rows[:, ch * C:(ch + 1) * C],
                    out_offset=None,
                    in_=fsh[:, :],
                    in_offset=bass.IndirectOffsetOnAxis(
                        ap=offs_t[:, ch:ch + 1], axis=0
                    ),
                )

            # transpose to channel-on-partition: G[64, g] = rows[g, ch]
            G = pool.tile([64, GCH * 128], f32)
            tp = ppool.tile([64, GCH * 128], f32, tag="tp")
            for ch in range(GCH):
                nc.tensor.transpose(
                    out=tp[:, ch * 128:(ch + 1) * 128],
                    in_=rows[:, ch * C:(ch + 1) * C],
                    identity=ident,
                )
            nc.vector.tensor_copy(G[:, :], tp[:, :])

            # stacked grams, 32x32 per block (rows/cols >= 26 are live-data
            # padding from neighboring pixel columns; never read back)
            S2 = ppool.tile([NRP, PB], f32, tag="S2")
            for bl in range(BPC):
                for pix in range(KK):
                    cb = (bl * KK + pix) * TWO_R
                    a = G[:, cb:cb + PB]
                    nc.tensor.matmul(
                        out=S2[bl * PB:(bl + 1) * PB, :], lhsT=a, rhs=a,
                        start=(pix == 0), stop=(pix == KK - 1),
                    )

            # row norms from the gram diagonal
            Ssb = pool.tile([NRP, PB], f32)
            nc.vector.tensor_copy(Ssb[:, :], S2[:, :])
            junk = pool.tile([NRP, PB], f32)
            d = pool.tile([NRP, 1], f32)
            nc.vector.tensor_tensor(out=junk, in0=Ssb, in1=mIm, op=Alu.mult)
            nc.vector.reduce_sum(d[:, :], junk[:, :], axis=mybir.AxisListType.X)
            sn = pool.tile([NRP, 1], f32)
            nc.scalar.sqrt(sn, d)
            snc = pool.tile([NRP, 1], f32)
            nc.vector.tensor_scalar_max(snc, sn, EPS)
            ri = pool.tile([NRP, 1], f32)
            nc.vector.reciprocal(ri, snc)
            # sim[m,n] = S[m,n]*rs_m*rs_n/tau. Column scaling + transpose in
            # one diagonal matmul per block (P2[m,n] = S[n,m]*rs_n), then a
            # row scaling by rs_m/tau on the DVE (S symmetric).
            Drs = pool.tile([NRP, PB], f32)
            nc.vector.tensor_scalar_mul(Drs, mIm, ri)
            P2 = ppool.tile([NRP, PB], f32, tag="P2")
            for bl in range(BPC):
                blk = slice(bl * PB, (bl + 1) * PB)
                nc.tensor.matmul(
                    out=P2[blk, :], lhsT=Ssb[blk, :], rhs=Drs[blk, :],
                    start=True, stop=True,
                )
            sim = pool.tile([NRP, PB], f32)
            nc.vector.tensor_scalar(
                out=sim, in0=P2[:, :], scalar1=ri, scalar2=float(1.0 / TAU),
                op0=Alu.mult, op1=Alu.mult,
            )
            # Z_i = sum_{j != i, j < 26} exp(sim_ij)
            E = pool.tile([NRP, PB], f32)
            nc.scalar.activation(E, sim, Act.Exp)
            ZJ = pool.tile([NRP, PB], f32)
            nc.vector.tensor_tensor(out=ZJ, in0=E, in1=mNotI, op=Alu.mult)
            Z = pool.tile([NRP, 1], f32)
            nc.vector.reduce_sum(Z[:, :], ZJ[:, :], axis=mybir.AxisListType.X)
            L = pool.tile([NRP, 1], f32)
            nc.scalar.activation(L, Z, Act.Ln)
            PJ = pool.tile([NRP, PB], f32)
            nc.vector.tensor_tensor(out=PJ, in0=sim, in1=mP, op=Alu.mult)
            pos = pool.tile([NRP, 1], f32)
            nc.vector.reduce_sum(pos[:, :], PJ[:, :], axis=mybir.AxisListType.X)
            lossv = pool.tile([NRP, 1], f32)
            nc.vector.tensor_tensor(out=lossv, in0=L, in1=pos, op=Alu.subtract)
            nc.sync.dma_start(out=lout[:, :], in_=lossv[:, :])
    nc.finalize()
    return nc


def kernel(f1, f2, b_idx, h_idx, w_idx):
    global LAST_RESULT
    from concourse.bass_utils import run_bass_kernel_spmd

    f1 = np.asarray(f1, dtype=np.float32)
    f2 = np.asarray(f2, dtype=np.float32)
    b_idx = np.asarray(b_idx).astype(np.int64)
    h_idx = np.asarray(h_idx).astype(np.int64)
    w_idx = np.asarray(w_idx).astype(np.int64)

    n = R * BS * KK
    j = np.arange(n)
    reg = j // (BS * KK)          # region of gather row j
    bpos = (j // KK) % BS         # positional output batch of row j
    pix = j % KK                  # pixel within block

    # which input batches does each core's gather touch?
    ship = []
    for c in range(NCORES):
        mask = (bpos // BPC) == c
        ship.append(np.unique(b_idx[mask]))
    nb = max(len(s) for s in ship)

    in_maps = []
    for c in range(NCORES):
        sb = ship[c]
        mask = (bpos // BPC) == c
        # local slot of each referenced input batch
        lslot = np.searchsorted(sb, b_idx[mask])
        bl = bpos[mask] % BPC
        px = pix[mask]
        rg = reg[mask]
        base = (lslot * H + h_idx[mask]) * W + w_idx[mask]
        offs = np.zeros(GCH * 128, np.int32)
        for s in range(2):  # feature side: f1 rows i<R, f2 rows i>=R
            g = (bl * KK + px) * TWO_R + s * R + rg
            offs[g] = base + s * nb * H * W
        fsh = np.zeros((2, nb, H * W, C), np.float32)
        fsh[0, : len(sb)] = f1[sb].reshape(len(sb), H * W, C)
        fsh[1, : len(sb)] = f2[sb].reshape(len(sb), H * W, C)
        in_maps.append(
            {
                "fsh": fsh.reshape(2 * nb * H * W, C),
                "offs": np.ascontiguousarray(offs.reshape(GCH, 128).T),
            }
        )

    key = nb
    if key not in _prog_cache:
        _prog_cache[key] = _build(nb)
    nc = _prog_cache[key]

    LAST_RESULT = run_bass_kernel_spmd(nc, in_maps, list(range(NCORES)))
    lv = np.concatenate(
        [r["lout"].reshape(-1)[bl * PB:bl * PB + TWO_R]
         for r in LAST_RESULT.results for bl in range(BPC)]
    )
    return np.float32(lv.mean())
